# revision 2
# baseline (speedup 1.0000x reference)
"""Fp8 per-token/per-channel quantized linear for Trainium2, 8 NeuronCores.

Computation (matches the jax reference):
    amax[m]  = max_k |x[m, k]|                       (x is bf16)
    xs[m]    = max(amax, 1e-10) / 448
    x_q      = e4m3fn_round(x / xs)                  (values up to +-448)
    out      = bf16((x_q @ W^T) * xs * w_scales) + bf16(bias)

Mapping to TRN2 hardware:
  * TRN's fp8 E4M3 saturates at +-240 (256..448 are Inf/NaN), so we quantize
    at HALF scale: x_q' = e4m3_round(x * (224/amax)) == x_q / 2 exactly (the
    fp8 grid is self-similar under powers of two), and fold the factor 2 into
    the output scale: out = psum * (amax/224) * w_scales.  The reference
    weights are already exactly fp8-representable, so casting them is lossless.
  * Sharding: row-parallel over M (8 cores x 1024 rows).  Each core quantizes
    only its own rows and streams the full weight, transposed on host to
    [K, N] tile layout and losslessly re-encoded to fp8.
  * x is transposed on-chip by the DMA XBAR (dma_start_transpose) straight
    into the [k_lo, k_sub, m] layout the DoubleRow GEMM wants -- the PE runs
    ONLY the 1024 main matmuls (no identity-transpose matmuls, no PSUM
    eviction copies).
  * The per-row quant scale inv=1/xs is broadcast across partitions with a
    tiny SBUF->DRAM->SBUF round trip (the DMA 0-stride broadcast pattern),
    then applied in transposed layout by DVE/GpSimd tensor_tensor multiplies
    (fp8 output), split half/half between the two engines.
  * Output stage is fused: one DVE scalar_tensor_tensor does
    (psum * xs[m]) * ws[n] -> bf16, GpSimd adds the (host pre-cast bf16)
    bias, SP-ring DMA stores.  w-scale/bias broadcasts are loaded per
    512-column block just in time.
  * Main GEMM runs in fp8 with perf_mode=DoubleRow (k=256 per matmul),
    all 8 PSUM banks in flight.
"""

import os
import numpy as np
import ml_dtypes
from contextlib import ExitStack

import concourse.bass as bass
import concourse.bacc as bacc
import concourse.tile as tile
from concourse import mybir
from concourse.bass_utils import run_bass_kernel_spmd

P = 128
M, K, N = 8192, 4096, 4096
NCORES = 8
M_SHARD = M // NCORES          # 1024 rows of x per core
M_TILES = M_SHARD // P         # 8
K_SUBS = K // P                # 32
K_HALF = K_SUBS // 2           # 16 ksubs per transpose/quantize half
K_SUPERS = K // (2 * P)        # 16 (DoubleRow consumes 256 rows of K)
N_BLK = 512
N_BLKS = N // N_BLK            # 8

FP8 = mybir.dt.float8e4
F32 = mybir.dt.float32
BF16 = mybir.dt.bfloat16

_PROGRAM_CACHE = {}


def _build_program():
    nc = bacc.Bacc(None, target_bir_lowering=False)

    x_d = nc.declare_dram_parameter("x", [M_SHARD, K], BF16, isOutput=False)
    # host layout: wt[nb, p, ksub, n] = weight[nb*512 + n, ksub*128 + p],
    # losslessly re-encoded to fp8 (reference weights are fp8-round-tripped)
    wt_d = nc.declare_dram_parameter("wt", [N_BLKS, P, K_SUBS, N_BLK], FP8, isOutput=False)
    ws_d = nc.declare_dram_parameter("ws", [N], F32, isOutput=False)
    bias_d = nc.declare_dram_parameter("bias", [N], BF16, isOutput=False)
    out_d = nc.declare_dram_parameter("out", [M_SHARD, N], BF16, isOutput=True)

    inv_scr = nc.dram_tensor("inv_scratch", [M_TILES, P], F32, kind="Internal")

    x_ap = x_d[:]
    wt_ap = wt_d[:]
    out_ap = out_d[:]

    with tile.TileContext(nc) as tc, ExitStack() as ctx:
        xtpool = ctx.enter_context(tc.tile_pool(name="xtpool", bufs=2))
        xTpool = ctx.enter_context(tc.tile_pool(name="xTpool", bufs=4))
        xqpool = ctx.enter_context(tc.tile_pool(name="xqpool", bufs=2 * M_TILES))
        wpool = ctx.enter_context(tc.tile_pool(name="wpool", bufs=4))
        stats = ctx.enter_context(tc.tile_pool(name="stats", bufs=8))
        xspool = ctx.enter_context(tc.tile_pool(name="xspool", bufs=M_TILES))
        invbpool = ctx.enter_context(tc.tile_pool(name="invbpool", bufs=3))
        wsbpool = ctx.enter_context(tc.tile_pool(name="wsbpool", bufs=3))
        biaspool = ctx.enter_context(tc.tile_pool(name="biaspool", bufs=3))
        opool = ctx.enter_context(tc.tile_pool(name="opool", bufs=6))
        psum_mm = ctx.enter_context(tc.tile_pool(name="psum_mm", bufs=8, space="PSUM"))

        wslab_tiles = [None] * N_BLKS
        wsb_tiles = [None] * N_BLKS
        bias_tiles = [None] * N_BLKS
        xs_tiles = [None] * M_TILES
        xq_half = [[None, None] for _ in range(M_TILES)]

        def issue_wslab(nb):
            t = wpool.tile([P, K_SUBS, N_BLK], FP8, tag="w")
            nc.scalar.dma_start(out=t[:], in_=wt_ap[nb])
            wslab_tiles[nb] = t

        def issue_wsb(nb):
            w = wsbpool.tile([P, N_BLK], F32, tag="wsb")
            nc.sync.dma_start(
                out=w[:],
                in_=bass.AP(tensor=ws_d[:].tensor, offset=nb * N_BLK,
                            ap=[[0, P], [1, N_BLK]]),
            )
            wsb_tiles[nb] = w
            b = biaspool.tile([P, N_BLK], BF16, tag="biasb")
            nc.sync.dma_start(
                out=b[:],
                in_=bass.AP(tensor=bias_d[:].tensor, offset=nb * N_BLK,
                            ap=[[0, P], [1, N_BLK]]),
            )
            bias_tiles[nb] = b

        def issue_chain(mt):
            # x row tile (ACT ring) -> amax -> scales (DVE)
            xt = xtpool.tile([P, K], BF16, tag="xt")
            nc.scalar.dma_start(out=xt[:], in_=x_ap[mt * P:(mt + 1) * P, :])

            amax = stats.tile([P, 1], F32, tag="amax")
            nc.vector.tensor_reduce(
                out=amax[:], in_=xt[:],
                axis=mybir.AxisListType.X, op=mybir.AluOpType.max,
                apply_absolute_value=True,
            )
            with tc.high_priority():
                xs = xspool.tile([P, 1], F32, tag="xs")
                nc.vector.tensor_scalar(
                    out=xs[:], in0=amax[:],
                    scalar1=1e-10, scalar2=1.0 / 224.0,
                    op0=mybir.AluOpType.max, op1=mybir.AluOpType.mult,
                )
                xs_tiles[mt] = xs
                inv = stats.tile([P, 1], F32, tag="inv")
                nc.vector.reciprocal(out=inv[:], in_=xs[:])

            # partition-broadcast of inv via DRAM round trip (SP ring)
            st1 = nc.sync.dma_start(out=inv_scr[mt], in_=inv[:])
            invb = invbpool.tile([P, P], F32, tag="invb")
            st2 = nc.sync.dma_start(
                out=invb[:],
                in_=bass.AP(tensor=inv_scr[:].tensor, offset=mt * P,
                            ap=[[0, P], [1, P]]),
            )
            tile.add_dep_helper(st2.ins, st1.ins, sync=True, reason="inv scratch raw")

            # XBAR transpose halves + quantize (DVE / GpSimd)
            ib = invb[:]
            in1 = bass.AP(tensor=ib.tensor, offset=ib.offset,
                          ap=[ib.ap[0], [0, K_HALF], ib.ap[1]])
            for h in range(2):
                xT = xTpool.tile([P, K_HALF, P], BF16, tag=f"xT{h}")
                nc.sync.dma_start_transpose(
                    out=xT[:], in_=xt[:, h * (K // 2):(h + 1) * (K // 2)])
                xq = xqpool.tile([P, K_HALF, P], FP8, tag=f"xq{h}")
                eng = nc.vector if h == 0 else nc.gpsimd
                eng.tensor_tensor(out=xq[:], in0=xT[:], in1=in1,
                                  op=mybir.AluOpType.mult)
                xq_half[mt][h] = xq

        # ---- preamble ----
        issue_wsb(0)
        issue_chain(0)
        issue_wslab(0)
        issue_chain(1)
        issue_chain(2)

        # ---- main fp8 DoubleRow GEMM, streamed over 512-col blocks of N ----
        wslab_sched = {(0, 5): 1, (0, 7): 2, (1, 1): 3, (1, 4): 4,
                       (2, 0): 5, (3, 0): 6, (4, 0): 7}
        chain_next = 3
        for nb in range(N_BLKS):
            if nb + 1 < N_BLKS:
                issue_wsb(nb + 1)
            wslab = wslab_tiles[nb]
            for mt in range(M_TILES):
                if (nb, mt) in wslab_sched:
                    issue_wslab(wslab_sched[(nb, mt)])
                pm = psum_mm.tile([P, N_BLK], F32, tag="pm")
                for j in range(K_SUPERS):
                    h, jj = divmod(j, 8)
                    nc.tensor.matmul(
                        out=pm[:],
                        lhsT=xq_half[mt][h][:, 2 * jj:2 * jj + 2, :],
                        rhs=wslab[:, 2 * j:2 * j + 2, :],
                        start=(j == 0), stop=(j == K_SUPERS - 1),
                        perf_mode=mybir.MatmulPerfMode.DoubleRow,
                    )
                sb = opool.tile([P, N_BLK], BF16, tag="sb")
                nc.vector.scalar_tensor_tensor(
                    out=sb[:], in0=pm[:], scalar=xs_tiles[mt][:],
                    in1=wsb_tiles[nb][:],
                    op0=mybir.AluOpType.mult, op1=mybir.AluOpType.mult,
                )
                nc.gpsimd.tensor_tensor(out=sb[:], in0=sb[:],
                                        in1=bias_tiles[nb][:],
                                        op=mybir.AluOpType.add)
                nc.sync.dma_start(
                    out=out_ap[mt * P:(mt + 1) * P, nb * N_BLK:(nb + 1) * N_BLK],
                    in_=sb[:],
                )
                if nb == 0 and chain_next < M_TILES:
                    issue_chain(chain_next)
                    chain_next += 1

    nc.compile()
    return nc


def _get_program():
    if "nc" not in _PROGRAM_CACHE:
        _PROGRAM_CACHE["nc"] = _build_program()
    return _PROGRAM_CACHE["nc"]


def _run_sharded(x, weight, weight_scales, bias, trace=False):
    x = np.asarray(x).astype(ml_dtypes.bfloat16, copy=False)
    weight = np.asarray(weight, dtype=np.float32)
    weight_scales = np.asarray(weight_scales, dtype=np.float32)
    bias16 = np.asarray(bias, dtype=np.float32).astype(ml_dtypes.bfloat16)

    # host-side sharding / layout only:
    # wt[nb, p, ksub, n] = weight[nb*512 + n, ksub*128 + p], re-encoded to
    # fp8 e4m3 (lossless: the reference weights are fp8-round-tripped values)
    wt = np.ascontiguousarray(
        weight.T.reshape(K_SUBS, P, N_BLKS, N_BLK).transpose(2, 1, 0, 3)
    ).astype(ml_dtypes.float8_e4m3)
    in_maps = []
    for c in range(NCORES):
        in_maps.append({
            "x": np.ascontiguousarray(x[c * M_SHARD:(c + 1) * M_SHARD]),
            "wt": wt,
            "ws": weight_scales,
            "bias": bias16,
        })

    nc = _get_program()
    res = run_bass_kernel_spmd(nc, in_maps, core_ids=list(range(NCORES)), trace=trace)
    out = np.concatenate([res.results[c]["out"] for c in range(NCORES)], axis=0)
    return out, res.exec_time_ns


def kernel(x, weight, weight_scales, bias):
    out, _ = _run_sharded(x, weight, weight_scales, bias,
                          trace=bool(os.environ.get("KERNEL_TRACE")))
    return out


# revision 9
# speedup vs baseline: 1.0307x; 1.0307x over previous
"""Fp8 per-token/per-channel quantized linear for Trainium2, 8 NeuronCores.

Computation (matches the jax reference):
    amax[m]  = max_k |x[m, k]|                       (x is bf16)
    xs[m]    = max(amax, 1e-10) / 448
    x_q      = e4m3fn_round(x / xs)                  (values up to +-448)
    out      = bf16((x_q @ W^T) * xs * w_scales) + bf16(bias)

Mapping to TRN2 hardware:
  * TRN's fp8 E4M3 saturates at +-240, so we quantize at HALF scale
    (factor folded into the output scale; exact for fp8's power-of-2 grid).
  * Sharding: row-parallel over M (8 cores x 1024 rows).
  * x is transposed on-chip by the DMA XBAR (dma_start_transpose) straight
    into [k_lo, k_sub, m] layout; quantize applies inv=1/xs in transposed
    layout (DVE 22 ksubs / GpSimd 10 ksubs, fp8 out), using a per-row scale
    broadcast via a tiny SBUF->DRAM->SBUF round trip.  PE runs ONLY the
    1024 DoubleRow matmuls.
  * Per-ring DMA bw is ~140GB/s and there are only 2 HWDGE rings (SP+ACT),
    so every large transfer (x tiles, weight slabs, outputs, transposes) is
    split across both rings, and the GEMM walks (nb, mt) in a rectangle-grow
    order so the early phase only needs a small corner of x-tiles x w-slabs.
  * Output stage fused: DVE scalar_tensor_tensor (psum*xs)*ws -> bf16,
    GpSimd adds host-precast bf16 bias, alternating-ring DMA out.
"""

import os
import numpy as np
import ml_dtypes
from contextlib import ExitStack

import concourse.bass as bass
import concourse.bacc as bacc
import concourse.tile as tile
from concourse import mybir
from concourse.bass_utils import run_bass_kernel_spmd

P = 128
M, K, N = 8192, 4096, 4096
NCORES = 8
M_SHARD = M // NCORES          # 1024 rows of x per core
M_TILES = M_SHARD // P         # 8
K_SUBS = K // P                # 32
KA = 22                        # ksubs quantized on DVE
KB = K_SUBS - KA               # ksubs quantized on GpSimd
K_SUPERS = K // (2 * P)        # 16 (DoubleRow consumes 256 rows of K)
N_BLK = 512
N_BLKS = N // N_BLK            # 8

FP8 = mybir.dt.float8e4
F32 = mybir.dt.float32
BF16 = mybir.dt.bfloat16

_PROGRAM_CACHE = {}


def _gemm_order():
    """Rectangle-grow (staircase) enumeration of (nb, mt), mt-biased 2:1."""
    order = [(0, 0)]
    nm, nn = 1, 1
    while nm < M_TILES or nn < N_BLKS:
        if nm < M_TILES and (nm < 2 * nn or nn == N_BLKS):
            order.extend((nb, nm) for nb in range(nn))
            nm += 1
        else:
            order.extend((nn, mt) for mt in range(nm))
            nn += 1
    return order


def _build_program():
    nc = bacc.Bacc(None, target_bir_lowering=False)

    x_d = nc.declare_dram_parameter("x", [M_SHARD, K], BF16, isOutput=False)
    # host layout: wt[nb, p, ksub, n] = weight[nb*512 + n, ksub*128 + p], fp8
    wt_d = nc.declare_dram_parameter("wt", [N_BLKS, P, K_SUBS, N_BLK], FP8, isOutput=False)
    ws_d = nc.declare_dram_parameter("ws", [N], F32, isOutput=False)
    bias_d = nc.declare_dram_parameter("bias", [N], BF16, isOutput=False)
    out_d = nc.declare_dram_parameter("out", [M_SHARD, N], BF16, isOutput=True)

    inv_scr = nc.dram_tensor("inv_scratch", [M_TILES, P], F32, kind="Internal")

    x_ap = x_d[:]
    wt_ap = wt_d[:]
    out_ap = out_d[:]

    with tile.TileContext(nc) as tc, ExitStack() as ctx:
        xtpool = ctx.enter_context(tc.tile_pool(name="xtpool", bufs=2))
        xTpoolA = ctx.enter_context(tc.tile_pool(name="xTpoolA", bufs=2))
        xTpoolB = ctx.enter_context(tc.tile_pool(name="xTpoolB", bufs=2))
        xqpoolA = ctx.enter_context(tc.tile_pool(name="xqpoolA", bufs=M_TILES))
        xqpoolB = ctx.enter_context(tc.tile_pool(name="xqpoolB", bufs=M_TILES))
        wpool = ctx.enter_context(tc.tile_pool(name="wpool", bufs=5))
        stats = ctx.enter_context(tc.tile_pool(name="stats", bufs=8))
        xspool = ctx.enter_context(tc.tile_pool(name="xspool", bufs=M_TILES))
        invbpool = ctx.enter_context(tc.tile_pool(name="invbpool", bufs=3))
        wsbpool = ctx.enter_context(tc.tile_pool(name="wsbpool", bufs=4))
        biaspool = ctx.enter_context(tc.tile_pool(name="biaspool", bufs=4))
        opool = ctx.enter_context(tc.tile_pool(name="opool", bufs=6))
        psum_mm = ctx.enter_context(tc.tile_pool(name="psum_mm", bufs=8, space="PSUM"))

        wslab_tiles = [None] * N_BLKS
        wsb_tiles = [None] * N_BLKS
        bias_tiles = [None] * N_BLKS
        xs_tiles = [None] * M_TILES
        xq_half = [[None, None] for _ in range(M_TILES)]

        def ring(i):
            return nc.sync if i % 2 == 0 else nc.scalar

        def issue_wslab(nb):
            # halves along ksub, one per ring (1MB each, contiguous)
            t = wpool.tile([P, K_SUBS, N_BLK], FP8, tag="w")
            nc.sync.dma_start(out=t[:, 0:K_SUBS // 2, :], in_=wt_ap[nb, :, 0:K_SUBS // 2, :])
            nc.scalar.dma_start(out=t[:, K_SUBS // 2:, :], in_=wt_ap[nb, :, K_SUBS // 2:, :])
            wslab_tiles[nb] = t

        def issue_wsb(nb):
            w = wsbpool.tile([P, N_BLK], F32, tag="wsb")
            ring(nb).dma_start(
                out=w[:],
                in_=bass.AP(tensor=ws_d[:].tensor, offset=nb * N_BLK,
                            ap=[[0, P], [1, N_BLK]]),
            )
            wsb_tiles[nb] = w
            b = biaspool.tile([P, N_BLK], BF16, tag="biasb")
            ring(nb + 1).dma_start(
                out=b[:],
                in_=bass.AP(tensor=bias_d[:].tensor, offset=nb * N_BLK,
                            ap=[[0, P], [1, N_BLK]]),
            )
            bias_tiles[nb] = b

        def issue_chain(mt):
            # x row tile: two pieces on both rings, split at the same ksub
            # boundary as the transpose pieces (each XBAR transpose then reads
            # a region written by exactly one DMA)
            xt = xtpool.tile([P, K], BF16, tag="xt")
            xsplit = KA * P
            xdma = [
                ring(mt).dma_start(out=xt[:, 0:xsplit],
                                   in_=x_ap[mt * P:(mt + 1) * P, 0:xsplit]),
                ring(mt + 1).dma_start(out=xt[:, xsplit:],
                                       in_=x_ap[mt * P:(mt + 1) * P, xsplit:]),
            ]

            amax = stats.tile([P, 1], F32, tag="amax")
            nc.vector.tensor_reduce(
                out=amax[:], in_=xt[:],
                axis=mybir.AxisListType.X, op=mybir.AluOpType.max,
                apply_absolute_value=True,
            )
            with tc.high_priority():
                xs = xspool.tile([P, 1], F32, tag="xs")
                nc.vector.tensor_scalar(
                    out=xs[:], in0=amax[:],
                    scalar1=1e-10, scalar2=1.0 / 224.0,
                    op0=mybir.AluOpType.max, op1=mybir.AluOpType.mult,
                )
                xs_tiles[mt] = xs
                inv = stats.tile([P, 1], F32, tag="inv")
                nc.vector.reciprocal(out=inv[:], in_=xs[:])

            # partition-broadcast of inv via DRAM round trip
            st1 = nc.sync.dma_start(out=inv_scr[mt], in_=inv[:])
            invb = invbpool.tile([P, P], F32, tag="invb")
            st2 = nc.sync.dma_start(
                out=invb[:],
                in_=bass.AP(tensor=inv_scr[:].tensor, offset=mt * P,
                            ap=[[0, P], [1, P]]),
            )
            tile.add_dep_helper(st2.ins, st1.ins, sync=True, reason="inv scratch raw")

            # XBAR transpose pieces + quantize (DVE: KA ksubs, GpSimd: KB)
            ib = invb[:]
            for h, (k0, kw) in enumerate([(0, KA), (KA, KB)]):
                xT = (xTpoolA if h == 0 else xTpoolB).tile([P, kw, P], BF16, tag=f"xT{h}")
                tr = nc.sync.dma_start_transpose(
                    out=xT[:], in_=xt[:, k0 * P:(k0 + kw) * P])
                tile.add_dep_helper(tr.ins, xdma[h].ins, sync=True,
                                    reason="transpose reads x piece")
                xq = (xqpoolA if h == 0 else xqpoolB).tile([P, kw, P], FP8, tag=f"xq{h}")
                in1 = bass.AP(tensor=ib.tensor, offset=ib.offset,
                              ap=[ib.ap[0], [0, kw], ib.ap[1]])
                eng = nc.vector if h == 0 else nc.gpsimd
                eng.tensor_tensor(out=xq[:], in0=xT[:], in1=in1,
                                  op=mybir.AluOpType.mult)
                xq_half[mt][h] = xq

        # ---- preamble ----
        issue_wsb(0)
        issue_chain(0)
        issue_wslab(0)
        issue_chain(1)
        issue_chain(2)
        issue_wslab(1)

        # ---- main GEMM: rectangle-grow order over (nb, mt) ----
        order = _gemm_order()
        pre_slot = {2: [("c", 3)], 4: [("w", 2)], 6: [("c", 4)],
                    11: [("c", 5), ("w", 3)], 16: [("c", 6)], 22: [("c", 7)],
                    24: [("w", 4)], 33: [("w", 5)], 41: [("w", 6)], 49: [("w", 7)]}
        wsb_done = {0}
        chain_next = 3
        for s, (nb, mt) in enumerate(order):
            for kind, idx in pre_slot.get(s, []):
                if kind == "c":
                    issue_chain(idx)
                else:
                    issue_wslab(idx)
            if nb not in wsb_done:
                issue_wsb(nb)
                wsb_done.add(nb)
            wslab = wslab_tiles[nb]
            pm = psum_mm.tile([P, N_BLK], F32, tag="pm")
            for j in range(K_SUPERS):
                if 2 * j + 2 <= KA:
                    lhsT = xq_half[mt][0][:, 2 * j:2 * j + 2, :]
                else:
                    lhsT = xq_half[mt][1][:, 2 * j - KA:2 * j - KA + 2, :]
                nc.tensor.matmul(
                    out=pm[:], lhsT=lhsT,
                    rhs=wslab[:, 2 * j:2 * j + 2, :],
                    start=(j == 0), stop=(j == K_SUPERS - 1),
                    perf_mode=mybir.MatmulPerfMode.DoubleRow,
                )
            sb = opool.tile([P, N_BLK], BF16, tag="sb")
            nc.vector.scalar_tensor_tensor(
                out=sb[:], in0=pm[:], scalar=xs_tiles[mt][:],
                in1=wsb_tiles[nb][:],
                op0=mybir.AluOpType.mult, op1=mybir.AluOpType.mult,
            )
            nc.gpsimd.tensor_tensor(out=sb[:], in0=sb[:],
                                    in1=bias_tiles[nb][:],
                                    op=mybir.AluOpType.add)
            ring(s).dma_start(
                out=out_ap[mt * P:(mt + 1) * P, nb * N_BLK:(nb + 1) * N_BLK],
                in_=sb[:],
            )

    nc.compile()
    return nc


def _get_program():
    if "nc" not in _PROGRAM_CACHE:
        _PROGRAM_CACHE["nc"] = _build_program()
    return _PROGRAM_CACHE["nc"]


def _run_sharded(x, weight, weight_scales, bias, trace=False):
    x = np.asarray(x).astype(ml_dtypes.bfloat16, copy=False)
    weight = np.asarray(weight, dtype=np.float32)
    weight_scales = np.asarray(weight_scales, dtype=np.float32)
    bias16 = np.asarray(bias, dtype=np.float32).astype(ml_dtypes.bfloat16)

    # host-side sharding / layout only (lossless fp8 re-encode of weights)
    wt = np.ascontiguousarray(
        weight.T.reshape(K_SUBS, P, N_BLKS, N_BLK).transpose(2, 1, 0, 3)
    ).astype(ml_dtypes.float8_e4m3)
    in_maps = []
    for c in range(NCORES):
        in_maps.append({
            "x": np.ascontiguousarray(x[c * M_SHARD:(c + 1) * M_SHARD]),
            "wt": wt,
            "ws": weight_scales,
            "bias": bias16,
        })

    nc = _get_program()
    res = run_bass_kernel_spmd(nc, in_maps, core_ids=list(range(NCORES)), trace=trace)
    out = np.concatenate([res.results[c]["out"] for c in range(NCORES)], axis=0)
    return out, res.exec_time_ns


def kernel(x, weight, weight_scales, bias):
    out, _ = _run_sharded(x, weight, weight_scales, bias,
                          trace=bool(os.environ.get("KERNEL_TRACE")))
    return out


# revision 12
# speedup vs baseline: 1.0377x; 1.0068x over previous
"""Fp8 per-token/per-channel quantized linear for Trainium2, 8 NeuronCores.

Computation (matches the jax reference):
    amax[m]  = max_k |x[m, k]|                       (x is bf16)
    xs[m]    = max(amax, 1e-10) / 448
    x_q      = e4m3fn_round(x / xs)                  (values up to +-448)
    out      = bf16((x_q @ W^T) * xs * w_scales) + bf16(bias)

Mapping to TRN2 hardware:
  * TRN's fp8 E4M3 saturates at +-240, so we quantize at HALF scale
    (factor folded into the output scale; exact on fp8's power-of-2 grid).
  * Sharding: row-parallel over M (8 cores x 1024 rows of x each); the full
    fp8-re-encoded weight streams through every core.
  * x is read from DRAM exactly once, via the DMA XBAR transpose
    (dma_start_transpose, SP ring only -- it corrupts data on the ACT ring)
    straight into the [k_lo, k_sub, m] layout the DoubleRow GEMM wants.
    There is no row-major x load at all: amax comes from the transposed
    pieces via a DVE abs_max tree + GpSimd partition_all_reduce(max), which
    also yields the quant scale already partition-broadcast (invb) with no
    DRAM round trip.  Only the output scale xs needs a tiny [1,128] ->
    DRAM -> [128,1] bounce.
  * Quantize: DVE multiplies piece A (16 ksubs), GpSimd piece B, fp8 out.
  * Ring budget (~125 GB/s per ring, 2 HWDGE rings): SP carries transposes
    (~8MB effective) + outputs (8MB) + scale bounces; ACT carries weights
    (16MB) + ws/bias block broadcasts.  Early weight slabs w1-w3 are split
    across both rings.  The GEMM walks (nb, mt) in rectangle-grow order so
    the early phase needs only a small corner of x-tiles x w-slabs.
  * Output stage fused: DVE scalar_tensor_tensor (psum*xs)*ws -> bf16,
    GpSimd adds host-precast bf16 bias, SP-ring DMA out.
  * PE runs ONLY the 1024 fp8 DoubleRow matmuls (k=256, n=512 each) with
    all 8 PSUM banks in flight.
"""

import os
import numpy as np
import ml_dtypes
from contextlib import ExitStack

import concourse.bass as bass
import concourse.bacc as bacc
import concourse.tile as tile
from concourse import mybir, bass_isa
from concourse.bass_utils import run_bass_kernel_spmd

P = 128
M, K, N = 8192, 4096, 4096
NCORES = 8
M_SHARD = M // NCORES          # 1024 rows of x per core
M_TILES = M_SHARD // P         # 8
K_SUBS = K // P                # 32
KH = K_SUBS // 2               # 16 ksubs per transpose/quantize piece
K_SUPERS = K // (2 * P)        # 16 (DoubleRow consumes 256 rows of K)
N_BLK = 512
N_BLKS = N // N_BLK            # 8

FP8 = mybir.dt.float8e4
F32 = mybir.dt.float32
BF16 = mybir.dt.bfloat16

_PROGRAM_CACHE = {}


def _gemm_order():
    """Rectangle-grow (staircase) enumeration of (nb, mt), mt-biased 2:1."""
    order = [(0, 0)]
    nm, nn = 1, 1
    while nm < M_TILES or nn < N_BLKS:
        if nm < M_TILES and (nm < 2 * nn or nn == N_BLKS):
            order.extend((nb, nm) for nb in range(nn))
            nm += 1
        else:
            order.extend((nn, mt) for mt in range(nm))
            nn += 1
    return order


def _build_program():
    nc = bacc.Bacc(None, target_bir_lowering=False)

    x_d = nc.declare_dram_parameter("x", [M_SHARD, K], BF16, isOutput=False)
    # host layout: wt[nb, p, ksub, n] = weight[nb*512 + n, ksub*128 + p], fp8
    wt_d = nc.declare_dram_parameter("wt", [N_BLKS, P, K_SUBS, N_BLK], FP8, isOutput=False)
    ws_d = nc.declare_dram_parameter("ws", [N], F32, isOutput=False)
    bias_d = nc.declare_dram_parameter("bias", [N], BF16, isOutput=False)
    out_d = nc.declare_dram_parameter("out", [M_SHARD, N], BF16, isOutput=True)

    xs_scr = nc.dram_tensor("xs_scratch", [M_TILES, P], F32, kind="Internal")

    x_ap = x_d[:]
    wt_ap = wt_d[:]
    out_ap = out_d[:]

    with tile.TileContext(nc) as tc, ExitStack() as ctx:
        xTpoolA = ctx.enter_context(tc.tile_pool(name="xTpoolA", bufs=2))
        xTpoolB = ctx.enter_context(tc.tile_pool(name="xTpoolB", bufs=2))
        xqpoolA = ctx.enter_context(tc.tile_pool(name="xqpoolA", bufs=M_TILES))
        xqpoolB = ctx.enter_context(tc.tile_pool(name="xqpoolB", bufs=M_TILES))
        wpool = ctx.enter_context(tc.tile_pool(name="wpool", bufs=5))
        tpabsA = ctx.enter_context(tc.tile_pool(name="tpabsA", bufs=2))
        tpabsB = ctx.enter_context(tc.tile_pool(name="tpabsB", bufs=2))
        tp8a = ctx.enter_context(tc.tile_pool(name="tp8a", bufs=2))
        tp8b = ctx.enter_context(tc.tile_pool(name="tp8b", bufs=2))
        tp8c = ctx.enter_context(tc.tile_pool(name="tp8c", bufs=2))
        tp4 = ctx.enter_context(tc.tile_pool(name="tp4", bufs=2))
        tp2 = ctx.enter_context(tc.tile_pool(name="tp2", bufs=2))
        tp1 = ctx.enter_context(tc.tile_pool(name="tp1", bufs=2))
        sbpool = ctx.enter_context(tc.tile_pool(name="sbpool", bufs=3))
        invbpool = ctx.enter_context(tc.tile_pool(name="invbpool", bufs=3))
        xspool = ctx.enter_context(tc.tile_pool(name="xspool", bufs=M_TILES))
        wsbpool = ctx.enter_context(tc.tile_pool(name="wsbpool", bufs=5))
        biaspool = ctx.enter_context(tc.tile_pool(name="biaspool", bufs=5))
        opool = ctx.enter_context(tc.tile_pool(name="opool", bufs=6))
        psum_mm = ctx.enter_context(tc.tile_pool(name="psum_mm", bufs=8, space="PSUM"))

        wslab_tiles = [None] * N_BLKS
        wsb_tiles = [None] * N_BLKS
        bias_tiles = [None] * N_BLKS
        xs_tiles = [None] * M_TILES
        xq_half = [[None, None] for _ in range(M_TILES)]

        def issue_wslab(nb, split):
            t = wpool.tile([P, K_SUBS, N_BLK], FP8, tag="w")
            if split:
                nc.scalar.dma_start(out=t[:, 0:KH, :], in_=wt_ap[nb, :, 0:KH, :])
                nc.sync.dma_start(out=t[:, KH:, :], in_=wt_ap[nb, :, KH:, :])
            else:
                nc.scalar.dma_start(out=t[:], in_=wt_ap[nb])
            wslab_tiles[nb] = t

        def issue_wsb(nb):
            w = wsbpool.tile([P, N_BLK], F32, tag="wsb")
            nc.scalar.dma_start(
                out=w[:],
                in_=bass.AP(tensor=ws_d[:].tensor, offset=nb * N_BLK,
                            ap=[[0, P], [1, N_BLK]]),
            )
            wsb_tiles[nb] = w
            b = biaspool.tile([P, N_BLK], BF16, tag="biasb")
            nc.scalar.dma_start(
                out=b[:],
                in_=bass.AP(tensor=bias_d[:].tensor, offset=nb * N_BLK,
                            ap=[[0, P], [1, N_BLK]]),
            )
            bias_tiles[nb] = b

        def issue_chain(mt):
            # XBAR transpose pieces straight from DRAM (SP ring only)
            xTs = []
            for h in range(2):
                xT = (xTpoolA if h == 0 else xTpoolB).tile([P, KH, P], BF16, tag=f"xT{h}")
                nc.sync.dma_start_transpose(
                    out=xT[:],
                    in_=x_ap[mt * P:(mt + 1) * P, h * (K // 2):(h + 1) * (K // 2)])
                xTs.append(xT)
            xTa, xTb = xTs

            # amax: ACT |x| (exact sign-clear), then DVE max tree
            abA = tpabsA.tile([P, KH, P], BF16, tag="abA")
            nc.scalar.activation(out=abA[:], in_=xTa[:], func=mybir.ActivationFunctionType.Abs)
            abB = tpabsB.tile([P, KH, P], BF16, tag="abB")
            nc.scalar.activation(out=abB[:], in_=xTb[:], func=mybir.ActivationFunctionType.Abs)
            m1 = tp8a.tile([P, 8, P], BF16, tag="m1")
            nc.vector.tensor_tensor(out=m1[:], in0=abA[:, 0:8, :], in1=abA[:, 8:16, :],
                                    op=mybir.AluOpType.max)
            m2 = tp8b.tile([P, 8, P], BF16, tag="m2")
            nc.vector.tensor_tensor(out=m2[:], in0=abB[:, 0:8, :], in1=abB[:, 8:16, :],
                                    op=mybir.AluOpType.max)
            m3 = tp8c.tile([P, 8, P], BF16, tag="m3")
            nc.vector.tensor_tensor(out=m3[:], in0=m1[:], in1=m2[:],
                                    op=mybir.AluOpType.max)
            m4 = tp4.tile([P, 4, P], BF16, tag="m4")
            nc.vector.tensor_tensor(out=m4[:], in0=m3[:, 0:4, :], in1=m3[:, 4:8, :],
                                    op=mybir.AluOpType.max)
            m5 = tp2.tile([P, 2, P], BF16, tag="m5")
            nc.vector.tensor_tensor(out=m5[:], in0=m4[:, 0:2, :], in1=m4[:, 2:4, :],
                                    op=mybir.AluOpType.max)
            m6 = tp1.tile([P, P], BF16, tag="m6")
            nc.vector.tensor_tensor(out=m6[:], in0=m5[:, 0, :], in1=m5[:, 1, :],
                                    op=mybir.AluOpType.max)

            # all-reduce across partitions -> amax[m] broadcast to every row
            allr = tp1.tile([P, P], F32, tag="allr")
            nc.gpsimd.partition_all_reduce(allr[:], m6[:], channels=P,
                                           reduce_op=bass_isa.ReduceOp.max)

            with tc.high_priority():
                xsb = sbpool.tile([P, P], F32, tag="xsb")
                nc.vector.tensor_scalar(
                    out=xsb[:], in0=allr[:],
                    scalar1=1e-10, scalar2=1.0 / 224.0,
                    op0=mybir.AluOpType.max, op1=mybir.AluOpType.mult,
                )
                invb = invbpool.tile([P, P], F32, tag="invb")
                nc.vector.reciprocal(out=invb[:], in_=xsb[:])

            # xs (partition-major, for the output stage) via tiny DRAM bounce
            st1 = nc.sync.dma_start(out=xs_scr[mt], in_=xsb[0:1, :])
            xs = xspool.tile([P, 1], F32, tag="xs")
            st2 = nc.sync.dma_start(
                out=xs[:],
                in_=bass.AP(tensor=xs_scr[:].tensor, offset=mt * P,
                            ap=[[1, P], [1, 1]]),
            )
            tile.add_dep_helper(st2.ins, st1.ins, sync=True, reason="xs scratch raw")
            xs_tiles[mt] = xs

            # quantize pieces: DVE for A, GpSimd for B (fp8 out)
            ib = invb[:]
            in1 = bass.AP(tensor=ib.tensor, offset=ib.offset,
                          ap=[ib.ap[0], [0, KH], ib.ap[1]])
            for h, xT in enumerate(xTs):
                xq = (xqpoolA if h == 0 else xqpoolB).tile([P, KH, P], FP8, tag=f"xq{h}")
                eng = nc.vector if h == 0 else nc.gpsimd
                eng.tensor_tensor(out=xq[:], in0=xT[:], in1=in1,
                                  op=mybir.AluOpType.mult)
                xq_half[mt][h] = xq

        # ---- preamble ----
        issue_wslab(0, split=False)
        issue_wsb(0)
        issue_chain(0)
        issue_wslab(1, split=True)
        issue_chain(1)
        issue_chain(2)

        # ---- main GEMM: rectangle-grow order over (nb, mt) ----
        order = _gemm_order()
        pre_slot = {1: [("c", 3)], 4: [("c", 4), ("w", 2)], 6: [("b", 2)],
                    8: [("c", 5)], 12: [("c", 6), ("w", 3)], 16: [("c", 7), ("b", 3)],
                    24: [("w", 4)], 30: [("b", 4)], 33: [("w", 5)], 38: [("b", 5)],
                    41: [("w", 6)], 46: [("b", 6)], 49: [("w", 7)], 54: [("b", 7)]}
        wsb_done = {0, 1}
        issue_wsb(1)
        for s, (nb, mt) in enumerate(order):
            for kind, idx in pre_slot.get(s, []):
                if kind == "c":
                    issue_chain(idx)
                elif kind == "w":
                    issue_wslab(idx, split=(idx <= 3))
                else:
                    issue_wsb(idx)
            wslab = wslab_tiles[nb]
            pm = psum_mm.tile([P, N_BLK], F32, tag="pm")
            for j in range(K_SUPERS):
                h, jj = divmod(j, 8)
                nc.tensor.matmul(
                    out=pm[:],
                    lhsT=xq_half[mt][h][:, 2 * jj:2 * jj + 2, :],
                    rhs=wslab[:, 2 * j:2 * j + 2, :],
                    start=(j == 0), stop=(j == K_SUPERS - 1),
                    perf_mode=mybir.MatmulPerfMode.DoubleRow,
                )
            sb = opool.tile([P, N_BLK], BF16, tag="sb")
            nc.vector.scalar_tensor_tensor(
                out=sb[:], in0=pm[:], scalar=xs_tiles[mt][:],
                in1=wsb_tiles[nb][:],
                op0=mybir.AluOpType.mult, op1=mybir.AluOpType.mult,
            )
            nc.gpsimd.tensor_tensor(out=sb[:], in0=sb[:],
                                    in1=bias_tiles[nb][:],
                                    op=mybir.AluOpType.add)
            nc.sync.dma_start(
                out=out_ap[mt * P:(mt + 1) * P, nb * N_BLK:(nb + 1) * N_BLK],
                in_=sb[:],
            )

    nc.compile()
    return nc


def _get_program():
    if "nc" not in _PROGRAM_CACHE:
        _PROGRAM_CACHE["nc"] = _build_program()
    return _PROGRAM_CACHE["nc"]


def _run_sharded(x, weight, weight_scales, bias, trace=False):
    x = np.asarray(x).astype(ml_dtypes.bfloat16, copy=False)
    weight = np.asarray(weight, dtype=np.float32)
    weight_scales = np.asarray(weight_scales, dtype=np.float32)
    bias16 = np.asarray(bias, dtype=np.float32).astype(ml_dtypes.bfloat16)

    # host-side sharding / layout only (lossless fp8 re-encode of weights)
    wt = np.ascontiguousarray(
        weight.T.reshape(K_SUBS, P, N_BLKS, N_BLK).transpose(2, 1, 0, 3)
    ).astype(ml_dtypes.float8_e4m3)
    in_maps = []
    for c in range(NCORES):
        in_maps.append({
            "x": np.ascontiguousarray(x[c * M_SHARD:(c + 1) * M_SHARD]),
            "wt": wt,
            "ws": weight_scales,
            "bias": bias16,
        })

    nc = _get_program()
    res = run_bass_kernel_spmd(nc, in_maps, core_ids=list(range(NCORES)), trace=trace)
    out = np.concatenate([res.results[c]["out"] for c in range(NCORES)], axis=0)
    return out, res.exec_time_ns


def kernel(x, weight, weight_scales, bias):
    out, _ = _run_sharded(x, weight, weight_scales, bias,
                          trace=bool(os.environ.get("KERNEL_TRACE")))
    return out
